# revision 9
# baseline (speedup 1.0000x reference)
"""Trainium2 Bass kernel for nn_Attention_8323646620215.

LayerNorm -> QKV -> scores(+rel-bias+mask) -> softmax -> attn@V -> out proj.

Sharding: 8 cores = (batch b in 0..3) x (query-half in 0..1). Each core
computes the full K/V for its batch and attention for its 1024 query rows;
no cross-core communication. Inside a core everything is computed in
transposed layouts:

  xn_T[feat, tok]  (PE transpose of the LN output)
  QT/KT[hd, tok] = Wqkv.T-slices @ xn_T      (scores contraction over hd=64)
  V[tok, hd]     = xn_T-slices.T @ Wqkv-v
  S_T[j, i]      = KT-slice.T @ QT           (psum, f32)
  P_T            = exp(S_T) * expA           (ACT exp, DVE mult)
  att_T[hd+1, i] = [V|1].T @ P_T             (row 64 = softmax denominator)
  y[i, :]        = sum_h outT_h-slice.T @ Wout-rows + bout

The mask+relative-bias enter multiplicatively: exp(s + bias + m) =
exp(s) * expA with expA = exp(clip-bias) * (mask != 0), fully precomputed
on the host as bf16 [2048, 1024] per core. LN gamma / attention scale /
beta are folded into the QKV weights host-side (exact transforms). x is
pre-cast to bf16 on the host (halves the critical input DMA).

Schedule: the activation engine runs only EXP during attention (its
~125us is the kernel floor). LayerNorm is a wide 3-stage pipeline over
DVE/Pool/ACT/PE; attention starts right after LN + a minimal K/Q/V
prefix; the remaining V tiles and later pairs' K/Q thread into the
attention loop's PE stream with due-date pacing. Softmax denominators
normalize through a DRAM-broadcast pipeline hidden inside the next
pair's iterations; the last pair splits its pipeline per ic-half so only
a short half remains exposed in the tail, where it overlaps the output
projection.

The host permutes each core's tokens so its own query half comes first,
which keeps the device program identical across cores (single NEFF).
"""
import sys
import types
import numpy as np

sys.path.insert(0, "/opt/trn_rl_repo")

# ---- environment fixes (axon agent container) -------------------------------
if "antenv.axon_hooks" not in sys.modules:
    _m = types.ModuleType("antenv.axon_hooks")
    _m._hook = None
    _m.set_axon_ntff_profile_hook = lambda h: setattr(_m, "_hook", h)
    _m.get_axon_ntff_profile_hook = lambda: _m._hook
    sys.modules["antenv.axon_hooks"] = _m
    try:
        from trn_agent_boot.trn_boot import _ntff_profile_via_ctypes
        _m._hook = _ntff_profile_via_ctypes("/opt/axon/libaxon_pjrt.so")
    except Exception:
        pass

import ml_dtypes  # noqa: E402
from concourse import bass, mybir, tile  # noqa: E402
from concourse.bass_utils import run_bass_kernel_spmd  # noqa: E402
from concourse.masks import make_identity  # noqa: E402

F32 = mybir.dt.float32
BF16 = mybir.dt.bfloat16
AF = mybir.ActivationFunctionType
OP = mybir.AluOpType

B, N, D, H, DH, MAXREL = 4, 2048, 512, 8, 64, 200
NQ = N // 2          # queries per core
NT = N // 128        # 16 token tiles
NCORES = 8

# This container's walrus rejects instructions with more than one sem wait.
# Splitting is sound: a same-engine NoOp right before the instruction
# enforces the wait at the same program point (sequencers run in order).


def _split_waits(nc, maxw=1):
    n_split = 0
    for f in nc.m.functions:
        for blk in f.blocks:
            bb = blk.bb if hasattr(blk, "bb") else blk
            insts = list(bb.instructions)
            out = []
            changed = False
            for inst in insts:
                si = inst.sync_info
                waits = list(si.on_wait) if si and si.on_wait else []
                if len(waits) > maxw:
                    extra = waits[:-maxw]
                    chunks = [extra[j:j + maxw] for j in range(0, len(extra), maxw)]
                    for i, chunk in enumerate(chunks):
                        nop = mybir.InstNoOp(name=f"{inst.name}-ws{i}", ins=[], outs=[])
                        nop.engine = inst.engine
                        nop.sync_info = mybir.SyncInfo(on_wait=chunk, on_update=[])
                        out.append(nop)
                    si.on_wait = waits[-maxw:]
                    changed = True
                    n_split += 1
                out.append(inst)
            if changed:
                bb.instructions = out
    return n_split


def build(has_c=False, has_b=False):
    nc = bass.Bass("TRN2", target_bir_lowering=False, debug=False,
                   num_devices=NCORES)
    x_d = nc.dram_tensor("x", [N, D], BF16, kind="ExternalInput")
    wqkv_d = nc.dram_tensor("wqkv", [D, 3 * D], BF16, kind="ExternalInput")
    cqkv_d = nc.dram_tensor("cqkv", [3 * D], F32, kind="ExternalInput")
    wout_d = nc.dram_tensor("wout", [D, D], BF16, kind="ExternalInput")
    bout_d = nc.dram_tensor("bout", [D], F32, kind="ExternalInput")
    expa_d = nc.dram_tensor("expa", [N, NQ], BF16, kind="ExternalInput")
    dsb_d = nc.dram_tensor("den_scratch", [H, NQ], BF16)
    dsi_d = nc.dram_tensor("invden_scratch", [H, NQ], BF16)
    y_d = nc.dram_tensor("y", [NQ, D], F32, kind="ExternalOutput")

    with tile.TileContext(nc) as tc, \
         tc.tile_pool(name="const", bufs=1) as C, \
         tc.tile_pool(name="pers", bufs=1) as P, \
         tc.tile_pool(name="work", bufs=3) as W:

        # ---- persistent tiles ----------------------------------------------
        xnT = P.tile([128, 4 * N], BF16, tag="xnT", name="xnT")  # [feat-blk|tok]
        KTp = [P.tile([128, N], BF16, tag=f"KT{hp}", name=f"KT{hp}") for hp in range(4)]
        QTp = [P.tile([128, NQ], BF16, tag=f"QT{hp}", name=f"QT{hp}") for hp in range(4)]
        Vau = [P.tile([128, H, 66], BF16, tag=f"V{t}", name=f"Vau{t}") for t in range(NT)]
        expA = [P.tile([128, NQ], BF16, tag=f"eA{t}", name=f"eA{t}") for t in range(NT)]
        numT = [P.tile([65, NQ], BF16, tag=f"nT{h}", name=f"nT{h}") for h in range(H)]
        pairT = [P.tile([128, NQ], BF16, tag=f"pT{hp}", name=f"pT{hp}") for hp in range(4)]
        x_ts = [P.tile([128, D], BF16, tag=f"x{t}", name=f"x{t}") for t in range(NT)]

        # ---- input DMAs ----------------------------------------------------
        # Three DGE queues in parallel: SP (x evens + expA evens), ACT (x
        # odds), gpsimd SWDGE (wqkv, expA odds, wout, consts). x tile 0 is
        # split in halves for the earliest possible LayerNorm start.
        nc.sync.dma_start(out=x_ts[0][:, 0:256], in_=x_d[0:128, 0:256])
        nc.sync.dma_start(out=x_ts[0][:, 256:512], in_=x_d[0:128, 256:512])
        nc.scalar.dma_start(out=x_ts[1][:, 0:256], in_=x_d[128:256, 0:256])
        nc.scalar.dma_start(out=x_ts[1][:, 256:512], in_=x_d[128:256, 256:512])
        for t in range(2, NT):
            eng = nc.sync if t % 2 == 0 else nc.scalar
            eng.dma_start(out=x_ts[t][:], in_=x_d[t * 128:(t + 1) * 128, :])
        wqkv_sb = [C.tile([128, 3 * D], BF16, tag=f"wqkv{kb}", name=f"wqkv{kb}") for kb in range(4)]
        for kb in range(4):
            nc.gpsimd.dma_start(out=wqkv_sb[kb][:],
                                in_=wqkv_d[kb * 128:(kb + 1) * 128, :])
        for t in range(0, NT, 2):
            nc.sync.dma_start(out=expA[t][:], in_=expa_d[t * 128:(t + 1) * 128, :])
        for t in range(1, NT, 2):
            nc.gpsimd.dma_start(out=expA[t][:], in_=expa_d[t * 128:(t + 1) * 128, :])
        woutP = [C.tile([128, D], BF16, tag=f"woutP{hp}", name=f"woutP{hp}") for hp in range(4)]
        for hp in range(4):
            nc.gpsimd.dma_start(out=woutP[hp][:],
                                in_=wout_d[hp * 128:(hp + 1) * 128, :])
        cq_all = C.tile([128, 12], F32, tag="cq")
        nc.gpsimd.dma_start(
            out=cq_all[:],
            in_=bass.AP(tensor=cqkv_d.ap().tensor, offset=0,
                        ap=[[1, 128], [128, 12]]))
        cv_bc = C.tile([128, D], F32, tag="cv")
        nc.gpsimd.dma_start(
            out=cv_bc[:],
            in_=bass.AP(tensor=cqkv_d.ap().tensor, offset=2 * D,
                        ap=[[0, 128], [1, D]]))
        bout_bc = C.tile([128, D], F32, tag="bout")
        nc.gpsimd.dma_start(
            out=bout_bc[:],
            in_=bass.AP(tensor=bout_d.ap().tensor, offset=0,
                        ap=[[0, 128], [1, D]]))

        ident = C.tile([128, 128], BF16, tag="ident")
        make_identity(nc, ident[:])
        eps_t = C.tile([128, 1], F32, tag="eps")
        nc.vector.memset(eps_t[:], 1e-5)
        for t in range(NT):
            nc.gpsimd.memset(Vau[t][:, :, 64:65], 1.0)

        # ---- Phase A: LayerNorm, wide pipeline -----------------------------
        # stats/aggr: Pool (even tiles) / DVE (odd); sqrt: ACT; recip (+ neg
        # mu*rs): DVE; apply: ACT (even) / DVE (odd); transpose: PE; drain:
        # one strided DVE copy per tile into the packed xnT tile.
        with tc.tile_pool(name="psA", bufs=2, space="PSUM") as psA:
            mvs, rss, nmr = [None] * NT, [None] * NT, [None] * NT
            for t in range(NT):
                st = W.tile([128, 6], BF16, tag="st", bufs=4)
                nc.vector.bn_stats(out=st[:], in_=x_ts[t][:])
                mv = W.tile([128, 2], F32, tag="mv", bufs=NT, name=f"mv{t}")
                nc.vector.bn_aggr(out=mv[:], in_=st[:])
                mvs[t] = mv
            for t in range(NT):
                rs = W.tile([128, 1], F32, tag="rs", bufs=NT, name=f"rs{t}")
                nc.scalar.activation(out=rs[:], in_=mvs[t][:, 1:2], func=AF.Sqrt,
                                     bias=eps_t[:])
                rss[t] = rs
            for t in range(NT):
                nc.vector.reciprocal(out=rss[t][:], in_=rss[t][:])
                nm = W.tile([128, 1], F32, tag="nmr", bufs=NT, name=f"nmr{t}")
                nc.vector.scalar_tensor_tensor(
                    out=nm[:], in0=mvs[t][:, 0:1], scalar=-1.0,
                    in1=rss[t][:], op0=OP.mult, op1=OP.mult)
                nmr[t] = nm
            for t in range(NT):
                xn_bf = W.tile([128, D], BF16, tag="xn", bufs=3)
                if t % 2 == 0:
                    nc.scalar.activation(out=xn_bf[:], in_=x_ts[t][:],
                                         func=AF.Identity, bias=nmr[t][:],
                                         scale=rss[t][:])
                else:
                    nc.vector.tensor_scalar(out=xn_bf[:], in0=x_ts[t][:],
                                            scalar1=mvs[t][:, 0:1],
                                            scalar2=rss[t][:],
                                            op0=OP.subtract, op1=OP.mult)
                tp = psA.tile([128, 512], BF16, tag="tr", bufs=2)
                for fb in range(4):
                    nc.tensor.transpose(tp[:, fb * 128:(fb + 1) * 128],
                                        xn_bf[:, fb * 128:(fb + 1) * 128],
                                        ident[:])
                xout = bass.AP(tensor=xnT[:].tensor,
                               offset=xnT[:].offset + t * 128,
                               ap=[xnT[:].ap[0], [N, 4], [1, 128]])
                if t % 2 == 0:
                    nc.vector.tensor_copy(out=xout, in_=tp[:])
                else:
                    nc.scalar.copy(out=xout, in_=tp[:])
            # preload the Exp activation table before the attention loop
            dummy_e = W.tile([128, 1], F32, tag="dume", bufs=1)
            nc.scalar.activation(out=dummy_e[:], in_=eps_t[:], func=AF.Exp)

        def xnT_k(kb, sl):
            a = xnT[:]
            return bass.AP(tensor=a.tensor, offset=a.offset + kb * N + sl.start,
                           ap=[a.ap[0], [1, sl.stop - sl.start]])

        # ---- Phase C: QKV production + attention ---------------------------
        # psum budget: sp [128,1024] x2 = 4 banks, av0/av1 [65,512] = 2,
        # qkv [128,512] x2 = 2.
        with tc.tile_pool(name="psC", bufs=1, space="PSUM") as psC:

            def make_k_chunk(hp, tc4, act_drain=False):
                def f():
                    kp = psC.tile([128, 512], F32, tag="qkv", bufs=2, name="kp")
                    for kb in range(4):
                        nc.tensor.matmul(
                            kp[:],
                            wqkv_sb[kb][:, D + hp * 128:D + (hp + 1) * 128],
                            xnT_k(kb, slice(tc4 * 512, (tc4 + 1) * 512)),
                            start=(kb == 0), stop=(kb == 3))
                    out = KTp[hp][:, tc4 * 512:(tc4 + 1) * 512]
                    if has_c:
                        nc.vector.tensor_scalar_add(
                            out=out, in0=kp[:],
                            scalar1=cq_all[:, 4 + hp:5 + hp])
                    elif act_drain:
                        nc.scalar.copy(out=out, in_=kp[:])
                    else:
                        nc.vector.tensor_copy(out=out, in_=kp[:])
                return f

            def make_q_chunk(hp, ic, act_drain=False):
                def f():
                    qp = psC.tile([128, 512], F32, tag="qkv", bufs=2, name="qp")
                    for kb in range(4):
                        nc.tensor.matmul(
                            qp[:],
                            wqkv_sb[kb][:, hp * 128:(hp + 1) * 128],
                            xnT_k(kb, slice(ic * 512, (ic + 1) * 512)),
                            start=(kb == 0), stop=(kb == 3))
                    out = QTp[hp][:, ic * 512:(ic + 1) * 512]
                    if has_c:
                        nc.vector.tensor_scalar_add(
                            out=out, in0=qp[:],
                            scalar1=cq_all[:, hp:hp + 1])
                    elif act_drain:
                        nc.scalar.copy(out=out, in_=qp[:])
                    else:
                        nc.vector.tensor_copy(out=out, in_=qp[:])
                return f

            def make_v_tile(t, act_drain=False):
                def f():
                    vp = psC.tile([128, 512], F32, tag="qkv", bufs=2, name="vp")
                    for kb in range(4):
                        nc.tensor.matmul(
                            vp[:],
                            xnT_k(kb, slice(t * 128, (t + 1) * 128)),
                            wqkv_sb[kb][:, 2 * D:3 * D],
                            start=(kb == 0), stop=(kb == 3))
                    if has_c:
                        nc.vector.tensor_add(out=Vau[t][:, :, 0:64], in0=vp[:],
                                             in1=cv_bc[:])
                    elif act_drain:
                        nc.scalar.copy(out=Vau[t][:, :, 0:64], in_=vp[:])
                    else:
                        nc.vector.tensor_copy(out=Vau[t][:, :, 0:64], in_=vp[:])
                return f

            # upfront prefix: K(pair0) chunk0, Q(pair0) ic0, V tiles 0-1
            make_k_chunk(0, 0)()
            make_q_chunk(0, 0)()
            make_v_tile(0)()
            make_v_tile(1)()

            # deferred PE work with due-iters (global iter = ic*16 + jt)
            def pair_extras(hp):
                ex = []
                if hp == 0:
                    ex += [(t - 1, make_v_tile(t, t % 2 == 0))
                           for t in range(2, NT)]
                    ex += [(4 * c - 2, make_k_chunk(0, c, c == 2))
                           for c in (1, 2, 3)]
                    ex += [(14, make_q_chunk(0, 1))]
                if hp < 3:
                    base = 2 if hp > 0 else 16
                    ex += [(base + 4 * c, make_k_chunk(hp + 1, c, c % 2 == 0))
                           for c in range(4)]
                    ex += [(min(base + 17, 29), make_q_chunk(hp + 1, 0, True)),
                           (min(base + 21, 31), make_q_chunk(hp + 1, 1))]
                ex.sort(key=lambda p: p[0])
                return ex

            # --- denominator/normalize pipeline (full width, hp0-2) ---------
            def den_pieces(hp):
                dal = W.tile([128, 2 * NQ // 128], BF16, tag="dall", bufs=2,
                             name=f"dal{hp}")
                dbs = [None, None]

                def p0():
                    nc.sync.dma_start(
                        out=dal[:],
                        in_=bass.AP(tensor=dsb_d.ap().tensor, offset=2 * hp * NQ,
                                    ap=[[2 * NQ // 128, 128], [1, 2 * NQ // 128]]))

                def p1():
                    nc.vector.tensor_scalar_add(out=dal[:], in0=dal[:],
                                                scalar1=1e-20)
                    with nc.allow_low_precision(reason="bf16 softmax denominators"):
                        nc.vector.reciprocal(out=dal[:], in_=dal[:])
                    nc.sync.dma_start(
                        out=bass.AP(tensor=dsi_d.ap().tensor, offset=2 * hp * NQ,
                                    ap=[[2 * NQ // 128, 128], [1, 2 * NQ // 128]]),
                        in_=dal[:])

                def load_bc(e):
                    def f():
                        h = 2 * hp + e
                        den_bc = W.tile([64, NQ], BF16, tag="denb", bufs=2,
                                        name=f"denb{h}")
                        dbs[e] = den_bc
                        nc.sync.dma_start(
                            out=den_bc[:],
                            in_=bass.AP(tensor=dsi_d.ap().tensor, offset=h * NQ,
                                        ap=[[0, 64], [1, NQ]]))
                    return f

                def mul_chunk(e, half):
                    def f():
                        h = 2 * hp + e
                        sl = slice(half * 512, (half + 1) * 512)
                        if e == 0:
                            nc.gpsimd.tensor_mul(out=pairT[hp][0:64, sl],
                                                 in0=numT[h][0:64, sl],
                                                 in1=dbs[e][:, sl])
                        else:
                            nc.gpsimd.tensor_mul(out=numT[h][0:64, sl],
                                                 in0=numT[h][0:64, sl],
                                                 in1=dbs[e][:, sl])
                    return f

                def stitch():
                    nc.sync.dma_start(out=pairT[hp][64:128, :],
                                      in_=numT[2 * hp + 1][0:64, :])

                return [p0, None, None, p1, load_bc(0), load_bc(1), None, None,
                        mul_chunk(0, 0), mul_chunk(0, 1), mul_chunk(1, 0),
                        mul_chunk(1, 1), stitch]

            # --- half-width denominator pipeline for hp3 --------------------
            def den_pieces_half(half):
                hp = 3
                sl = slice(half * 512, (half + 1) * 512)
                dals = [None, None]
                dbs = [None, None]

                def p0(e):
                    def f():
                        h = 2 * hp + e
                        dal = W.tile([128, 4], BF16, tag="dalh", bufs=4,
                                     name=f"dalh{e}_{half}")
                        dals[e] = dal
                        nc.sync.dma_start(
                            out=dal[:],
                            in_=bass.AP(tensor=dsb_d.ap().tensor,
                                        offset=h * NQ + half * 512,
                                        ap=[[4, 128], [1, 4]]))
                    return f

                def p1(e):
                    def f():
                        h = 2 * hp + e
                        dal = dals[e]
                        nc.vector.tensor_scalar_add(out=dal[:], in0=dal[:],
                                                    scalar1=1e-20)
                        with nc.allow_low_precision(reason="bf16 softmax denominators"):
                            nc.vector.reciprocal(out=dal[:], in_=dal[:])
                        nc.sync.dma_start(
                            out=bass.AP(tensor=dsi_d.ap().tensor,
                                        offset=h * NQ + half * 512,
                                        ap=[[4, 128], [1, 4]]),
                            in_=dal[:])
                    return f

                def load_bc(e):
                    def f():
                        h = 2 * hp + e
                        den_bc = W.tile([64, 512], BF16, tag="denbh", bufs=4,
                                        name=f"denbh{e}_{half}")
                        dbs[e] = den_bc
                        nc.sync.dma_start(
                            out=den_bc[:],
                            in_=bass.AP(tensor=dsi_d.ap().tensor,
                                        offset=h * NQ + half * 512,
                                        ap=[[0, 64], [1, 512]]))
                    return f

                def mul(e):
                    def f():
                        h = 2 * hp + e
                        if e == 0:
                            nc.gpsimd.tensor_mul(out=pairT[hp][0:64, sl],
                                                 in0=numT[h][0:64, sl],
                                                 in1=dbs[e][:, :])
                        else:
                            nc.gpsimd.tensor_mul(out=numT[h][0:64, sl],
                                                 in0=numT[h][0:64, sl],
                                                 in1=dbs[e][:, :])
                    return f

                def stitch():
                    nc.sync.dma_start(out=pairT[hp][64:128, sl],
                                      in_=numT[2 * hp + 1][0:64, sl])

                return [p0(0), p0(1), p1(0), p1(1), load_bc(0), load_bc(1),
                        None, mul(0), mul(1), stitch]

            pend = []

            # --- attention main loop: hp outer, ic mid, jt inner ------------
            for hp in range(4):
                h0, h1 = 2 * hp, 2 * hp + 1
                exq = pair_extras(hp)

                for ic in range(2):
                    i5 = ic * 512
                    av0 = psC.tile([65, 512], F32, tag="av0", name="av0", bufs=1)
                    av1 = psC.tile([65, 512], F32, tag="av1", name="av1", bufs=1)
                    for jt in range(NT):
                        gi = ic * NT + jt
                        sp = psC.tile([128, 1024], F32, tag="sp", bufs=2)
                        nc.tensor.matmul(
                            sp[:, 0:512],
                            KTp[hp][0:64, jt * 128:(jt + 1) * 128],
                            QTp[hp][0:64, i5:i5 + 512],
                            start=True, stop=True, tile_position=(0, 0))
                        nc.tensor.matmul(
                            sp[:, 512:1024],
                            KTp[hp][64:128, jt * 128:(jt + 1) * 128],
                            QTp[hp][64:128, i5:i5 + 512],
                            start=True, stop=True, tile_position=(64, 0))
                        while exq and exq[0][0] <= gi:
                            exq.pop(0)[1]()
                        eb = W.tile([128, 2, 512], BF16, tag="eb", bufs=3)
                        nc.scalar.activation(out=eb[:, :, :], in_=sp[:],
                                             func=AF.Exp)
                        pb = W.tile([128, 2, 512], BF16, tag="pb", bufs=6)
                        ea_bc = bass.AP(
                            tensor=expA[jt][:].tensor,
                            offset=expA[jt][:].offset + i5,
                            ap=[expA[jt][:].ap[0], [0, 2], [1, 512]])
                        nc.vector.tensor_mul(out=pb[:, :, :], in0=eb[:, :, :],
                                             in1=ea_bc)
                        nc.tensor.matmul(av0[:], Vau[jt][:, h0, 0:65],
                                         pb[:, 0, :],
                                         start=(jt == 0), stop=(jt == NT - 1))
                        nc.tensor.matmul(av1[:], Vau[jt][:, h1, 0:65],
                                         pb[:, 1, :],
                                         start=(jt == 0), stop=(jt == NT - 1))
                        if pend and gi >= 2:
                            fcl = pend.pop(0)
                            if fcl is not None:
                                fcl()
                    # drain av psum -> numT columns for this ic
                    nc.vector.tensor_copy(out=numT[h0][:, i5:i5 + 512],
                                          in_=av0[:])
                    nc.vector.tensor_copy(out=numT[h1][:, i5:i5 + 512],
                                          in_=av1[:])
                    if hp == 3:
                        # launch the half-width denominator pipe immediately
                        for e in range(2):
                            nc.sync.dma_start(
                                out=dsb_d[2 * hp + e, i5:i5 + 512],
                                in_=numT[2 * hp + e][64:65, i5:i5 + 512])
                        if ic == 0:
                            pend = den_pieces_half(0)
                while exq:
                    exq.pop(0)[1]()
                if hp < 3:
                    for e in range(2):
                        h = 2 * hp + e
                        nc.sync.dma_start(out=dsb_d[h, :], in_=numT[h][64:65, :])
                    pend = den_pieces(hp)

            # hp3-ic1 half pipe: runs at the end of phase C (exposed tail)
            for fcl in pend:          # any hp3-ic0 leftovers
                if fcl is not None:
                    fcl()
            for fcl in den_pieces_half(1):
                if fcl is not None:
                    fcl()

        # ---- Phase D: output projection (head pairs, K=128) ----------------
        with tc.tile_pool(name="psD", bufs=1, space="PSUM") as psD:
            yps = [psD.tile([128, 512], F32, tag=f"yp{isl}", name=f"yp{isl}")
                   for isl in range(8)]
            for hp in range(4):
                for isl in range(8):
                    nc.tensor.matmul(yps[isl][:],
                                     pairT[hp][:, isl * 128:(isl + 1) * 128],
                                     woutP[hp][:],
                                     start=(hp == 0), stop=(hp == 3))
                    if hp == 3:
                        ysb = W.tile([128, 512], F32, tag="ysb", bufs=4)
                        if has_b:
                            nc.vector.tensor_add(out=ysb[:], in0=yps[isl][:],
                                                 in1=bout_bc[:])
                        elif isl % 2 == 0:
                            nc.vector.tensor_copy(out=ysb[:], in_=yps[isl][:])
                        else:
                            nc.scalar.copy(out=ysb[:], in_=yps[isl][:])
                        yeng = nc.sync if isl % 2 == 0 else nc.scalar
                        yeng.dma_start(
                            out=y_d[isl * 128:(isl + 1) * 128, :], in_=ysb[:])
    _split_waits(nc)
    return nc


_NC_CACHE = {}


def _get_nc(has_c, has_b):
    key = (has_c, has_b)
    if key not in _NC_CACHE:
        _NC_CACHE[key] = build(has_c, has_b)
    return _NC_CACHE[key]


LAST_EXEC_TIME_NS = None


def kernel(x, gamma, beta, Wqkv, Wout, bout, rel_table, temporal_mask,
           trace=True):
    global LAST_EXEC_TIME_NS
    x = np.asarray(x, np.float32)
    gamma = np.asarray(gamma, np.float32)
    beta = np.asarray(beta, np.float32)
    Wqkv = np.asarray(Wqkv, np.float32)
    Wout = np.asarray(Wout, np.float32)
    bout = np.asarray(bout, np.float32)
    rel_table = np.asarray(rel_table, np.float32)
    temporal_mask = np.asarray(temporal_mask)

    scale = DH ** -0.5
    w_eff = (Wqkv * gamma[:, None]).copy()
    w_eff[:, :D] *= scale
    cqkv = (beta @ Wqkv).astype(np.float32)
    cqkv[:D] *= scale
    wqkv_bf = w_eff.astype(ml_dtypes.bfloat16)
    wout_bf = Wout.astype(ml_dtypes.bfloat16)

    # expA[j, i] = exp(rel_bias[i - j]) * (mask[i, j] != 0)  (key j, query i)
    idx = np.arange(N)
    rel = np.clip(idx[None, :] - idx[:, None], -(MAXREL - 1), MAXREL - 1) \
        + MAXREL - 1  # [j, i] -> clip(i - j)
    expA_full = np.exp(rel_table[rel]) * (temporal_mask.T != 0)  # [j, i] f32

    keyperm_half = [
        np.concatenate([np.arange(i0, i0 + NQ),
                        np.arange(NQ - i0, NQ - i0 + NQ)])
        for i0 in (0, NQ)
    ]
    expa_half = [
        np.ascontiguousarray(
            expA_full[keyperm_half[half]][:, half * NQ:(half + 1) * NQ]
        ).astype(ml_dtypes.bfloat16)
        for half in range(2)
    ]

    in_maps = []
    for c in range(NCORES):
        b, half = c // 2, c % 2
        xp = np.ascontiguousarray(x[b][keyperm_half[half]]).astype(
            ml_dtypes.bfloat16)
        in_maps.append({
            "x": xp,
            "wqkv": wqkv_bf,
            "cqkv": cqkv,
            "wout": wout_bf,
            "bout": bout,
            "expa": expa_half[half],
        })

    nc = _get_nc(bool(np.any(cqkv != 0.0)), bool(np.any(bout != 0.0)))
    res = run_bass_kernel_spmd(nc, in_maps, core_ids=list(range(NCORES)),
                               trace=trace)
    LAST_EXEC_TIME_NS = res.exec_time_ns

    out = np.empty((B, N, D), np.float32)
    for c in range(NCORES):
        b, half = c // 2, c % 2
        out[b, half * NQ:(half + 1) * NQ] = res.results[c]["y"]
    return out
